# revision 5
# baseline (speedup 1.0000x reference)
"""Trainium2 Bass kernel for nn_InputAverageModel (segment_reduce).

Computation (reference semantics):
  x = data_seq[..., 0]                          # [B, T, N]
  gmean = mean of valid (!= -1) entries of x    # scalar
  x_imputed = where(valid, x, gmean)
  pred_speed = tile(mean_t(x_imputed), H)       # [B, H, N]
  regional   = per-cluster mean over N          # [B, H, R]

Strategy: data-parallel over batch (16 -> 2 per core across 8 cores).
The memory-bound pass (reading data_seq once) runs on device: per
(b, n) it produces S_z = sum_t relu(x) (sum of valid values, exact
since valid values are >= 0 and invalid ones are exactly -1) and
n_inv = sum_t (x == -1) (exact count, computed in bf16 0/1 then
accumulated in fp32 PSUM).  The T-axis (partition) reduction is a
ones-vector matmul on the TensorEngine; value stream stays fp32 for
precision, count stream is bf16 (exact for 0/1).  The tiny cross-core
scalar reduction (gmean) plus the O(B*N) epilogue (imputation-adjusted
time mean, horizon tiling, 16-way segment means) run on host.
"""

import numpy as np

B, T, N, F = 16, 288, 4096, 2
H, R = 10, 16
N_CORES = 8
BPC = B // N_CORES  # batch elements per core
NULL = -1.0
CHUNKS = [(0, 128), (128, 128), (256, 32)]  # T = 128 + 128 + 32

_CACHE = {}


def _build_nc():
    import concourse.tile as tile
    from concourse import bacc, mybir

    f32 = mybir.dt.float32
    bf16 = mybir.dt.bfloat16

    nc = bacc.Bacc("TRN2", target_bir_lowering=False, debug=False,
                   num_devices=N_CORES)
    x = nc.dram_tensor("x", [BPC, T, N, F], f32, kind="ExternalInput").ap()
    out = nc.dram_tensor("out", [2 * BPC, N], f32, kind="ExternalOutput").ap()

    with tile.TileContext(nc) as tc:
        with (
            tc.tile_pool(name="xp", bufs=3) as xp,
            tc.tile_pool(name="zp", bufs=2) as zp,
            tc.tile_pool(name="mp", bufs=2) as mp,
            tc.tile_pool(name="cn", bufs=1) as cn,
            tc.tile_pool(name="op", bufs=1) as op,
            tc.tile_pool(name="pp", bufs=1, space="PSUM") as pp,
        ):
            ones_f = cn.tile([128, 1], f32)
            nc.vector.memset(ones_f[:], 1.0)
            ones_b = cn.tile([128, 1], bf16)
            nc.vector.memset(ones_b[:], 1.0)

            # One PSUM bank-set: group g at partition 32*g, block j in bank j.
            # g = 2*b + q with q=0 -> S_z (values), q=1 -> n_inv (counts).
            ps = pp.tile([128, 4096], f32)

            osb = op.tile([128, 4096], f32)
            for b in range(BPC):
                gz = (2 * b) * 32
                gm = (2 * b + 1) * 32
                for ci, (t0, rows) in enumerate(CHUNKS):
                    start = ci == 0
                    stop = ci == len(CHUNKS) - 1
                    xt = xp.tile([128, 8192], f32, tag="xt")
                    nc.sync.dma_start(
                        out=xt[:rows],
                        in_=x[b, t0:t0 + rows].rearrange("t n f -> t (n f)"),
                    )
                    xe = xt[:rows, 0:8192:2]  # feature 0 (even columns)
                    zt = zp.tile([128, 4096], f32, tag="zt")
                    nc.scalar.activation(zt[:rows], xe,
                                         mybir.ActivationFunctionType.Relu)
                    mt = mp.tile([128, 4096], bf16, tag="mt")
                    nc.vector.tensor_scalar(mt[:rows], xe, NULL, None,
                                            mybir.AluOpType.is_equal)
                    for j in range(8):
                        sl = slice(j * 512, (j + 1) * 512)
                        nc.tensor.matmul(ps[gz:gz + 1, sl], ones_f[:rows],
                                         zt[:rows, sl], start=start, stop=stop,
                                         tile_position=(0, gz))
                        nc.tensor.matmul(ps[gm:gm + 1, sl], ones_b[:rows],
                                         mt[:rows, sl], start=start, stop=stop,
                                         tile_position=(0, gm))
                # batch b's accumulation is complete; copy out its two rows
                # (overlaps the next batch's DMA/compute)
                nc.scalar.copy(osb[gz:gz + 1, :], ps[gz:gz + 1, :])
                nc.vector.tensor_copy(osb[gm:gm + 1, :], ps[gm:gm + 1, :])
            for g in range(2 * BPC):
                nc.sync.dma_start(out=out[g:g + 1, :],
                                  in_=osb[32 * g:32 * g + 1, :])

    nc.compile()
    return nc


def _get_nc():
    if "nc" not in _CACHE:
        _CACHE["nc"] = _build_nc()
    return _CACHE["nc"]


def _run_device(x):
    """x: [16, 288, 4096, 2] f32 -> per-(b,n) S_z and n_inv, each [16, 4096]."""
    from concourse.bass_utils import run_bass_kernel_spmd

    nc = _get_nc()
    in_maps = [
        {"x": np.ascontiguousarray(x[BPC * i:BPC * (i + 1)])}
        for i in range(N_CORES)
    ]
    res = run_bass_kernel_spmd(nc, in_maps, core_ids=list(range(N_CORES)))
    outs = np.stack([res.results[i]["out"] for i in range(N_CORES)])  # [8,4,N]
    s_z = outs[:, 0::2, :].reshape(B, N)
    n_inv = outs[:, 1::2, :].reshape(B, N)
    return s_z, n_inv


def kernel(data_seq, cluster_id):
    x = np.asarray(data_seq, dtype=np.float32)
    cid = np.asarray(cluster_id).astype(np.int64)
    assert x.shape == (B, T, N, F), x.shape

    s_z, n_inv = _run_device(x)
    s_z = s_z.astype(np.float64)
    n_inv = n_inv.astype(np.float64)

    total_sum = s_z.sum()
    total_valid = (T - n_inv).sum()
    gmean = total_sum / max(total_valid, 1.0)

    mean_t = (s_z + n_inv * gmean) / T                     # [B, N]
    pred_speed = np.broadcast_to(
        mean_t.astype(np.float32)[:, None, :], (B, H, N)
    ).copy()

    counts = np.bincount(cid, minlength=R).astype(np.float64)   # [R]
    onehot = (cid[:, None] == np.arange(R)[None, :]).astype(np.float64)
    seg = mean_t @ onehot                                   # [B, R]
    regional_bh = (seg / np.maximum(counts, 1.0)[None, :]).astype(np.float32)
    regional = np.broadcast_to(regional_bh[:, None, :], (B, H, R)).copy()

    return pred_speed, regional
